# revision 20
# baseline (speedup 1.0000x reference)
"""Trainium2 Bass kernel for GQA attention (B=2, S=2048, D=1024, H=16, HKV=4).

Sharding: 8 cores = batch(2) x kv-group(4). Each core handles one batch and
one KV head group (4 query heads + 1 KV head), computes attention plus its
partial slice of the output projection (row-parallel wo); the host sums the
4 tensor-parallel partials per batch (partials are bf16, summed in f32).
No device collectives.

Per-core device kernel (matmul operands bf16, fp32 PSUM accumulation),
organized as four 512-wide q subpasses that pipeline end to end:
  1. Stage 1 per group g (4 s-blocks): QKV projection with xT group-slices
     streamed from DRAM, RoPE on DVE (weights pre-permuted so each head is
     [32 real | 32 imag]), PE-transpose q -> qT and k -> kT2 (k duplicated to
     partitions 64-127 for two-head row tiling).
  2. Immediately after group g: scores S^T = kT.T @ qT for all k-blocks
     kb <= 4g+3 over q in [512g, 512(g+1)), both heads of a pair in one
     [128, 2w] PSUM tile; diagonal blocks masked by accumulating
     identity @ (-1e9 upper-triangle); one exp per chunk on ACT
     (scale=1/8 folded; no max-subtraction, masked entries exp(-1e9)=0)
     into per-(pair,kb,subpass) eSb slots in two parity-rotated buffers.
  3. Transposed PV per q-block: po[q=128, 2*65] accumulates
     eSb_block[k,q].T @ v[k, 65] over kb <= qb, with a ones column in v
     collecting the softmax denominator per q ROW (col 64/129).
  4. Normalize = DVE reciprocal [128,2] + tensor_scalar per head (per-
     partition scalar broadcast), then PE-transpose [q,hd] -> attnT[hd,q]
     and Pool copy to SBUF.
  5. y = attnT.T @ woT per s-block as soon as both pairs' q-block is
     normalized; copies to SBUF on DVE/Pool alternately; DMA out.
  PV/normalize/stage3 for subpass sp are emitted between later subpasses'
  score chunks so PE and ACT (~58us of irreducible exp) stay busy together.
"""

import numpy as np
import ml_dtypes

B, S, D = 2, 2048, 1024
H, HKV, HD = 16, 4, 64
REP = H // HKV  # 4 query heads per kv head
N_CORES = 8
NSB = S // 128  # 16 s-blocks
NDC = D // 128  # 8 d-chunks
QKV = REP * HD + 2 * HD  # 384 projected dims per core
NSP = 4  # q subpasses of 512
BF16 = ml_dtypes.bfloat16

_CACHE = {}
DEBUG = False


def _w_of(kb, sp):
    """q-width of the (kb, sp) score chunk (per head)."""
    return (sp + 1) * 512 - max(sp * 512, 128 * kb)


def _spoff():
    """eSb slot offsets: off[sp][(pair, kb)] in the single resident buffer."""
    offs = []
    cum = 0
    for sp in range(NSP):
        off = {}
        for pair in range(2):
            for kb in range(4 * sp + 4):
                off[(pair, kb)] = cum
                cum += 2 * _w_of(kb, sp)
        offs.append(off)
    return offs, cum


SPOFF, EBUF_SIZE = _spoff()


def _build_module():
    from contextlib import ExitStack

    import concourse.bacc as bacc
    import concourse.mybir as mybir
    import concourse.tile as tile
    from concourse.alu_op_type import AluOpType

    f32 = mybir.dt.float32
    bf16 = mybir.dt.bfloat16
    Exp = mybir.ActivationFunctionType.Exp
    mult, add, sub = AluOpType.mult, AluOpType.add, AluOpType.subtract

    nc = bacc.Bacc("TRN2", target_bir_lowering=False, debug=False,
                   num_devices=N_CORES)

    xT_d = nc.dram_tensor("xT", (D, S), bf16, kind="ExternalInput").ap()
    wcat_d = nc.dram_tensor("wcatT", (D, QKV), bf16, kind="ExternalInput").ap()
    woT_d = nc.dram_tensor("woT", (2 * 128, D), bf16, kind="ExternalInput").ap()
    ctq_d = nc.dram_tensor("ctq", (128, S), bf16, kind="ExternalInput").ap()
    stq_d = nc.dram_tensor("stq", (128, S), bf16, kind="ExternalInput").ap()
    ctk_d = nc.dram_tensor("ctk", (128, NSB * 32), bf16, kind="ExternalInput").ap()
    stk_d = nc.dram_tensor("stk", (128, NSB * 32), bf16, kind="ExternalInput").ap()
    mneg_d = nc.dram_tensor("maskneg", (128, 128), bf16, kind="ExternalInput").ap()
    idn_d = nc.dram_tensor("ident", (128, 128), bf16, kind="ExternalInput").ap()
    y_d = nc.dram_tensor("y", (S, D), bf16, kind="ExternalOutput").ap()
    if DEBUG:
        esd_d = nc.dram_tensor("esd", (128, EBUF_SIZE), bf16,
                               kind="ExternalOutput").ap()
        att_d = nc.dram_tensor("attd", (128, 2 * S), bf16,
                               kind="ExternalOutput").ap()

    with tile.TileContext(nc) as tc:
        with ExitStack() as ctx:
            persist = ctx.enter_context(tc.tile_pool(name="persist", bufs=1))
            woT = persist.tile([128, 2 * D], bf16)       # 4 KB
            mneg = persist.tile([128, 128], bf16)
            idn = persist.tile([128, 128], bf16)
            qT = persist.tile([128, 2 * S], bf16)        # 8 KB
            kT2 = persist.tile([128, S], bf16)           # 4 KB
            vb = persist.tile([128, NSB * 65], bf16)     # ~2 KB (v + ones col)
            eS = persist.tile([128, EBUF_SIZE], bf16, name="eS")
            dummy = persist.tile([1, 8], f32)

            psS = ctx.enter_context(
                tc.tile_pool(name="psS", bufs=2, space="PSUM"))
            recp = ctx.enter_context(tc.tile_pool(name="recip", bufs=2))
            anat = ctx.enter_context(tc.tile_pool(name="anat", bufs=2))
            yst = ctx.enter_context(tc.tile_pool(name="ystage", bufs=4))

            s1 = ctx.enter_context(ExitStack())
            s1p = s1.enter_context(tc.tile_pool(name="s1p", bufs=1))
            wcat = s1p.tile([128, NDC * QKV], bf16)  # 6 KB
            ctq = s1p.tile([128, S], bf16)
            stq = s1p.tile([128, S], bf16)
            ctk = s1p.tile([128, NSB * 32], bf16)
            stk = s1p.tile([128, NSB * 32], bf16)
            xtp = s1.enter_context(tc.tile_pool(name="xtp", bufs=2))
            qkvp = s1.enter_context(tc.tile_pool(name="qkvp", bufs=2))
            psq = s1.enter_context(
                tc.tile_pool(name="psqkv", bufs=2, space="PSUM"))
            tpq = s1.enter_context(
                tc.tile_pool(name="tpq", bufs=1, space="PSUM"))
            tpk = s1.enter_context(
                tc.tile_pool(name="tpk", bufs=1, space="PSUM"))
            tmp = s1.enter_context(tc.tile_pool(name="ropetmp", bufs=1))

            # DMA order tuned for the group-0 critical chain: x/w chunks
            # first (proj can start after chunk 0), tables sliced per group
            def dma_xtg(xTg, g):
                nc.sync.dma_start(
                    xTg[:].rearrange("p (dc s) -> p dc s", dc=NDC),
                    xT_d.rearrange("(dc p) s -> p dc s", dc=NDC)[
                        :, :, g * 512:(g + 1) * 512])

            xTg0 = xtp.tile([128, NDC * 512], bf16, tag="xtg", name="xtg0")
            nc.sync.dma_start(
                xTg0[:, 0:1024].rearrange("p (dc s) -> p dc s", dc=2),
                xT_d.rearrange("(dc p) s -> p dc s", dc=NDC)[
                    :, 0:2, 0:512])
            nc.sync.dma_start(
                wcat[:, 0:2 * QKV].rearrange("p (dc q) -> p dc q", dc=2),
                wcat_d.rearrange("(dc p) q -> p dc q", dc=NDC)[:, 0:2, :])
            nc.sync.dma_start(
                xTg0[:, 1024:].rearrange("p (dc s) -> p dc s", dc=6),
                xT_d.rearrange("(dc p) s -> p dc s", dc=NDC)[
                    :, 2:8, 0:512])
            nc.sync.dma_start(
                wcat[:, 2 * QKV:].rearrange("p (dc q) -> p dc q", dc=6),
                wcat_d.rearrange("(dc p) q -> p dc q", dc=NDC)[:, 2:8, :])
            nc.sync.dma_start(ctq[:, 0:512], ctq_d[:, 0:512])
            nc.sync.dma_start(stq[:, 0:512], stq_d[:, 0:512])
            nc.sync.dma_start(ctk[:], ctk_d[:])
            nc.sync.dma_start(stk[:], stk_d[:])
            nc.sync.dma_start(idn[:], idn_d[:])
            nc.sync.dma_start(mneg[:], mneg_d[:])
            nc.sync.dma_start(ctq[:, 512:S], ctq_d[:, 512:S])
            nc.sync.dma_start(stq[:, 512:S], stq_d[:, 512:S])
            nc.sync.dma_start(
                woT[:].rearrange("p (c d) -> p c d", c=2),
                woT_d.rearrange("(c p) d -> p c d", c=2))
            nc.gpsimd.memset(vb[:], 1.0)
            # warm the ACT exp table while DMAs run
            nc.gpsimd.memset(dummy[:], 0.0)
            nc.scalar.activation(dummy[:], dummy[:], Exp)

            # ---- stage 1 (per 4-s-block group): proj + rope + transposes --
            # qkv group layout: col = sbl*320 + h*64 + half*32 + j  (q)
            #                   col = sbl*320 + 256 + half*32 + j   (k)
            def emit_group(g, xTg=None, on_act=False):
                if xTg is None:
                    xTg = xtp.tile([128, NDC * 512], bf16, tag="xtg",
                                   name=f"xtg{g}")
                    dma_xtg(xTg, g)
                qkv = qkvp.tile([128, 4 * 320], bf16, tag="qkv",
                                name=f"qkv{g}")
                for sbl in range(4):
                    sb = 4 * g + sbl
                    ps = psq.tile([128, QKV], f32, tag="ps", name=f"ps{sb}")
                    for dc in range(NDC):
                        nc.tensor.matmul(
                            ps[:],
                            lhsT=xTg[:, dc * 512 + sbl * 128:
                                     dc * 512 + (sbl + 1) * 128],
                            rhs=wcat[:, dc * QKV:(dc + 1) * QKV],
                            start=(dc == 0), stop=(dc == NDC - 1))
                    if on_act:
                        nc.scalar.copy(
                            qkv[:, sbl * 320: sbl * 320 + 320], ps[:, 0:320])
                        nc.scalar.copy(
                            vb[:, sb * 65: sb * 65 + 64], ps[:, 320:384])
                    else:
                        nc.vector.tensor_copy(
                            qkv[:, sbl * 320: sbl * 320 + 320], ps[:, 0:320])
                        nc.vector.tensor_copy(
                            vb[:, sb * 65: sb * 65 + 64], ps[:, 320:384])

                # rope for group g (4 s-blocks at once), in place
                g4 = qkv[:].rearrange("p (sbl x) -> p sbl x", sbl=4)
                qg = g4[:, :, 0:256].rearrange("p sbl (h c) -> p sbl h c",
                                               c=64)
                qr, qi = qg[:, :, :, 0:32], qg[:, :, :, 32:64]
                kg = g4[:, :, 256:320]
                kr, ki = kg[:, :, 0:32], kg[:, :, 32:64]
                ct = ctq[:, g * 512:(g + 1) * 512].rearrange(
                    "p (sbl h j) -> p sbl h j", sbl=4, h=REP)
                st = stq[:, g * 512:(g + 1) * 512].rearrange(
                    "p (sbl h j) -> p sbl h j", sbl=4, h=REP)
                ctks = ctk[:, g * 128:(g + 1) * 128].rearrange(
                    "p (sbl j) -> p sbl j", sbl=4)
                stks = stk[:, g * 128:(g + 1) * 128].rearrange(
                    "p (sbl j) -> p sbl j", sbl=4)
                tA = tmp.tile([128, 512], bf16, tag="tA", name=f"tA{g}")
                tB = tmp.tile([128, 512], bf16, tag="tB", name=f"tB{g}")
                tC = tmp.tile([128, 512], bf16, tag="tC", name=f"tC{g}")
                tD = tmp.tile([128, 512], bf16, tag="tD", name=f"tD{g}")
                r3 = lambda t: t[:].rearrange("p (sbl h j) -> p sbl h j",
                                              sbl=4, h=REP)
                nc.vector.tensor_tensor(r3(tA), qr, ct, mult)
                nc.vector.tensor_tensor(r3(tB), qi, st, mult)
                nc.vector.tensor_tensor(r3(tC), qr, st, mult)
                nc.vector.tensor_tensor(r3(tD), qi, ct, mult)
                nc.vector.tensor_tensor(qr, r3(tA), r3(tB), sub)
                nc.vector.tensor_tensor(qi, r3(tC), r3(tD), add)
                tE = tmp.tile([128, 128], bf16, tag="tE", name=f"tE{g}")
                tF = tmp.tile([128, 128], bf16, tag="tF", name=f"tF{g}")
                tG = tmp.tile([128, 128], bf16, tag="tG", name=f"tG{g}")
                tH = tmp.tile([128, 128], bf16, tag="tH", name=f"tH{g}")
                r2 = lambda t: t[:].rearrange("p (sbl j) -> p sbl j", sbl=4)
                nc.vector.tensor_tensor(r2(tE), kr, ctks, mult)
                nc.vector.tensor_tensor(r2(tF), ki, stks, mult)
                nc.vector.tensor_tensor(r2(tG), kr, stks, mult)
                nc.vector.tensor_tensor(r2(tH), ki, ctks, mult)
                nc.vector.tensor_tensor(kr, r2(tE), r2(tF), sub)
                nc.vector.tensor_tensor(ki, r2(tG), r2(tH), add)

                # transposes: q -> qT, k -> kT2[0:64]
                for half in range(2):  # sbl pairs (0,1) and (2,3)
                    pt = tpq.tile([128, 512], bf16, tag="ptq",
                                  name=f"ptq{g}_{half}")
                    for li, (sbl, hb) in enumerate(
                            [(2 * half, 0), (2 * half, 1),
                             (2 * half + 1, 0), (2 * half + 1, 1)]):
                        src = qkv[:, sbl * 320 + hb * 128:
                                  sbl * 320 + hb * 128 + 128]
                        nc.tensor.transpose(
                            pt[:, li * 128:(li + 1) * 128], src, idn[:])
                    dst = qT[:].rearrange(
                        "p (hb sb c) -> p sb hb c", hb=2, sb=NSB)[
                        :, 4 * g + 2 * half: 4 * g + 2 * half + 2, :, :]
                    nc.vector.tensor_copy(
                        dst, pt[:].rearrange("p (sb hb c) -> p sb hb c",
                                             sb=2, hb=2))
                ptk = tpk.tile([64, 512], bf16, tag="ptk", name=f"ptk{g}")
                for sbl in range(4):
                    nc.tensor.transpose(
                        ptk[:, sbl * 128:(sbl + 1) * 128],
                        qkv[:, sbl * 320 + 256: sbl * 320 + 320],
                        idn[:])
                nc.vector.tensor_copy(
                    kT2[0:64, g * 512:(g + 1) * 512], ptk[:])
                nc.sync.dma_start(kT2[64:128, g * 512:(g + 1) * 512],
                                  kT2[0:64, g * 512:(g + 1) * 512])

            # ---- stage 2: scores + exp per (pair, kb, subpass) chunk ----
            def emit_chunk(pair, kb, sp):
                w = _w_of(kb, sp)
                qlo = max(sp * 512, 128 * kb)
                qhi = (sp + 1) * 512
                diag = 128 * kb == qlo
                pp = psS.tile([128, 1024], f32, tag="pp",
                              name=f"pp{pair}_{kb}_{sp}")
                for i in range(2):
                    # head i region starts at i*512: PSUM zero-regions are
                    # 2KB-bank granular, so each group must be bank-aligned
                    nc.tensor.matmul(
                        pp[:, i * 512: i * 512 + w],
                        lhsT=kT2[i * 64:(i + 1) * 64,
                                 kb * 128:(kb + 1) * 128],
                        rhs=qT[i * 64:(i + 1) * 64,
                               pair * S + qlo: pair * S + qhi],
                        start=True, stop=not diag)
                    if diag:
                        nc.tensor.matmul(
                            pp[:, i * 512: i * 512 + 128],
                            lhsT=idn[:], rhs=mneg[:],
                            start=False, stop=True,
                            skip_group_check=True)
                off = SPOFF[sp][(pair, kb)]
                src = pp[:].rearrange("p (h c) -> p h c", h=2)[:, :, 0:w]
                dst = eS[:, off: off + 2 * w].rearrange(
                    "p (h c) -> p h c", h=2)
                nc.scalar.activation(dst, src, Exp, scale=0.125)

            # ---- stage 3/4: PV + normalize + transpose per q-block ----
            def emit_pv(pair, qb, pvpool, ptpool):
                sp = qb // 4
                po = pvpool.tile([128, 130], f32, tag="po",
                                 name=f"po{pair}_{qb}")
                # serialize heads: interleaving start=True groups in one
                # bank wipes the other group's pending-zero bytes
                for i in range(2):
                    for kb in range(qb + 1):
                        w = _w_of(kb, sp)
                        qlo = max(sp * 512, 128 * kb)
                        col0 = SPOFF[sp][(pair, kb)] + qb * 128 - qlo
                        nc.tensor.matmul(
                            po[:, i * 65:(i + 1) * 65],
                            lhsT=eS[:, col0 + i * w:
                                    col0 + i * w + 128],
                            rhs=vb[:, kb * 65: kb * 65 + 65],
                            start=(kb == 0), stop=(kb == qb))
                rc = recp.tile([128, 2], f32, tag="rc",
                               name=f"rc{pair}_{qb}")
                nc.vector.reciprocal(
                    rc[:], po[:].rearrange("p (h c) -> p h c",
                                           h=2)[:, :, 64])
                an = anat.tile([128, 128], bf16, tag="an",
                               name=f"an{pair}_{qb}")
                for i in range(2):
                    nc.vector.tensor_scalar(
                        an[:, i * 64:(i + 1) * 64],
                        po[:, i * 65: i * 65 + 64],
                        rc[:, i:i + 1], None, mult)
                pt = ptpool.tile([128, 128], bf16, tag="pt",
                                 name=f"pt{pair}_{qb}")
                nc.tensor.transpose(pt[:], an[:], idn[:])
                nc.vector.tensor_copy(
                    attnT[pair][:, qb * 128:(qb + 1) * 128], pt[:])

            def emit_stage3(sb):
                yp = psS.tile([128, D], f32, tag="pp", name=f"yp{sb}")
                for hp in range(2):
                    for c2 in range(2):
                        nc.tensor.matmul(
                            yp[:, c2 * 512:(c2 + 1) * 512],
                            lhsT=attnT[hp][:, sb * 128:(sb + 1) * 128],
                            rhs=woT[:, hp * D + c2 * 512:
                                    hp * D + (c2 + 1) * 512],
                            start=(hp == 0), stop=(hp == 1))
                ys = yst.tile([128, D], bf16, tag="ys", name=f"ys{sb}")
                nc.vector.tensor_copy(ys[:], yp[:])
                nc.sync.dma_start(y_d[sb * 128:(sb + 1) * 128, :], ys[:])

            # ---- schedule ----
            # Parity buffers: sp0/sp2 share eA, sp1/sp3 share eB, so PV(sp)
            # must be emitted before S(sp+2) chunks overwrite its buffer.
            def Sc(sp, kbs):
                for pair in range(2):
                    for kb in kbs:
                        emit_chunk(pair, kb, sp)

            def PV(sp, pvpool, ptpool):
                for pair in range(2):
                    for qb in range(4 * sp, 4 * sp + 4):
                        emit_pv(pair, qb, pvpool, ptpool)

            emit_group(0, xTg0, on_act=True)
            Sc(0, range(4))
            emit_group(1, on_act=True)
            Sc(1, range(8))
            emit_group(2, on_act=True)
            Sc(2, range(4))
            emit_group(3, on_act=True)
            Sc(3, range(16))
            s1.close()
            pvpool = ctx.enter_context(
                tc.tile_pool(name="pv", bufs=3, space="PSUM"))
            ptpool = ctx.enter_context(
                tc.tile_pool(name="pt", bufs=1, space="PSUM"))
            attp = ctx.enter_context(tc.tile_pool(name="attp", bufs=1))
            attnT = [attp.tile([128, S], bf16, tag="attnT0", name="attnT0"),
                     attp.tile([128, S], bf16, tag="attnT1", name="attnT1")]
            # interleaved worklist: one S2 chunk, then one q-block's
            # PV(both pairs) + stage3; sp2 q-blocks last so the tail is
            # a single short q-block chain
            qb_order = [0, 1, 2, 3, 4, 5, 6, 7, 12, 13, 14, 15, 8, 9, 10, 11]
            s2_chunks = [(pair, kb) for kb in range(4, 12) for pair in range(2)]
            for i, qb in enumerate(qb_order):
                pair, kb = s2_chunks[i]
                emit_chunk(pair, kb, 2)
                emit_pv(0, qb, pvpool, ptpool)
                emit_pv(1, qb, pvpool, ptpool)
                if i >= 2:
                    emit_stage3(qb_order[i - 2])
            for i in (14, 15):
                emit_stage3(qb_order[i])
            if DEBUG:
                nc.sync.dma_start(esd_d[:], eS[:])
                for hp in range(2):
                    nc.sync.dma_start(att_d[:, hp * S:(hp + 1) * S],
                                      attnT[hp][:])

    nc.compile()
    return nc


def _get_module():
    if "nc" not in _CACHE:
        _CACHE["nc"] = _build_module()
    return _CACHE["nc"]


def _host_tables(freqs_cos, freqs_sin):
    # ctq[p, sb*128 + h*32 + j] = cos[sb*128 + p, j]  (tiled over 4 heads)
    c3 = freqs_cos.reshape(NSB, 128, 32).transpose(1, 0, 2)  # [p, sb, j]
    s3 = freqs_sin.reshape(NSB, 128, 32).transpose(1, 0, 2)
    ctq = np.broadcast_to(c3[:, :, None, :],
                          (128, NSB, REP, 32)).reshape(128, S)
    stq = np.broadcast_to(s3[:, :, None, :],
                          (128, NSB, REP, 32)).reshape(128, S)
    # ctk[p, sb*32 + j] = cos[sb*128 + p, j]
    ctk = np.ascontiguousarray(c3).reshape(128, NSB * 32)
    stk = np.ascontiguousarray(s3).reshape(128, NSB * 32)
    return ctq, stq, ctk, stk


def make_in_maps(x, wq, wk, wv, wo, freqs_cos, freqs_sin):
    x = np.asarray(x, np.float32)
    wq = np.asarray(wq, np.float32)
    wk = np.asarray(wk, np.float32)
    wv = np.asarray(wv, np.float32)
    wo = np.asarray(wo, np.float32)
    freqs_cos = np.asarray(freqs_cos, np.float32)
    freqs_sin = np.asarray(freqs_sin, np.float32)

    # deinterleave rope pairs within each head: [r0 i0 r1 i1 ...] ->
    # [r0..r31 | i0..i31]
    idx = np.concatenate([np.arange(0, HD, 2), np.arange(1, HD, 2)])
    wq_p = wq.reshape(H, HD, D)[:, idx, :].reshape(H * HD, D)
    wk_p = wk.reshape(HKV, HD, D)[:, idx, :].reshape(HKV * HD, D)

    ctq, stq, ctk, stk = _host_tables(freqs_cos, freqs_sin)
    kk, qq = np.arange(128)[:, None], np.arange(128)[None, :]
    maskneg = np.where(kk <= qq, 0.0, -1e9).astype(np.float32)
    ident = np.eye(128)

    common = {
        "ctq": ctq.astype(BF16), "stq": stq.astype(BF16),
        "ctk": ctk.astype(BF16), "stk": stk.astype(BF16),
        "maskneg": maskneg.astype(BF16), "ident": ident.astype(BF16),
    }
    xT_b = [np.ascontiguousarray(x[b].T).astype(BF16) for b in range(B)]
    in_maps = []
    for core in range(N_CORES):
        b, g = divmod(core, HKV)
        wqT = wq_p[g * 256:(g + 1) * 256].T
        wkT = wk_p[g * 64:(g + 1) * 64].T
        wvT = wv[g * 64:(g + 1) * 64].T
        wcat = np.ascontiguousarray(
            np.concatenate([wqT, wkT, wvT], axis=1)).astype(BF16)
        woTg = np.ascontiguousarray(wo[:, g * 256:(g + 1) * 256].T).astype(BF16)
        in_maps.append({"xT": xT_b[b], "wcatT": wcat, "woT": woTg, **common})
    return in_maps


def _causal_fast_path_ok(mask):
    m = np.asarray(mask)
    if m.shape != (S, S):
        return False
    upper = m[np.triu_indices(S, 1)]
    lower = m[np.tril_indices(S, 0)]
    return bool(np.all(upper <= -1e8) and np.all(lower == 0))


def _numpy_fallback(x, wq, wk, wv, wo, freqs_cos, freqs_sin, mask):
    x = np.asarray(x, np.float32)
    xq = (x.reshape(B * S, D) @ np.asarray(wq, np.float32).T).reshape(B, S, H, HD)
    xk = (x.reshape(B * S, D) @ np.asarray(wk, np.float32).T).reshape(B, S, HKV, HD)
    xv = (x.reshape(B * S, D) @ np.asarray(wv, np.float32).T).reshape(B, S, HKV, HD)

    def rope(t, nh):
        tf = t.reshape(B, S, nh, HD // 2, 2)
        tr, ti = tf[..., 0], tf[..., 1]
        c = np.asarray(freqs_cos, np.float32)[None, :, None, :]
        s = np.asarray(freqs_sin, np.float32)[None, :, None, :]
        outr = tr * c - ti * s
        outi = tr * s + ti * c
        return np.stack([outr, outi], axis=-1).reshape(B, S, nh, HD)

    xq = rope(xq, H)
    xk = rope(xk, HKV)
    xqg = xq.reshape(B, S, HKV, REP, HD)
    scores = np.einsum("bqgrd,bkgd->bgrqk", xqg, xk) / np.sqrt(np.float32(HD))
    scores = scores + np.asarray(mask, np.float32)[None, None, None, :, :]
    scores = scores - scores.max(axis=-1, keepdims=True)
    e = np.exp(scores)
    attn = e / e.sum(axis=-1, keepdims=True)
    out = np.einsum("bgrqk,bkgd->bqgrd", attn, xv).reshape(B, S, H * HD)
    return (out.reshape(B * S, H * HD) @ np.asarray(wo, np.float32)
            .T).reshape(B, S, D).astype(np.float32)


def kernel(x, wq, wk, wv, wo, freqs_cos, freqs_sin, mask):
    if not _causal_fast_path_ok(mask):
        return _numpy_fallback(x, wq, wk, wv, wo, freqs_cos, freqs_sin, mask)
    from concourse import bass_utils
    nc = _get_module()
    in_maps = make_in_maps(x, wq, wk, wv, wo, freqs_cos, freqs_sin)
    res = bass_utils.run_bass_kernel_spmd(nc, in_maps,
                                          core_ids=list(range(N_CORES)))
    y = np.zeros((B, S, D), np.float32)
    for core in range(N_CORES):
        b = core // HKV
        y[b] += res.results[core]["y"].astype(np.float32)
    return y
